# revision 14
# baseline (speedup 1.0000x reference)
"""Trainium2 Bass kernel for nn_AttentionLayer (DIN-style attention scoring MLP).

Math (per batch b, key position s):
    feats = [q, k, q*k, q-k]                       # [4E] = 256
    h1 = relu(feats @ W0 + b0)                     # 128
    h2 = relu(h1 @ W1 + b1)                        # 64
    score = h2 @ W2 + b2                           # scalar
    attn = softmax_s(score masked to s < len[b])
    out = sum_s attn[s] * k[s]                     # [E]

Algebraic refactor (host folds weights, all exact):
    W0 rows: [0:64]=Wq(q), [64:128]=Wkk(k), [128:192]=Wc(q*k), [192:256]=Wd(q-k)
    h1 = relu(q@(Wq+Wd) + k@(Wkk-Wd) + (q*k)@Wc + b0)

Device layout: feature-major ("transposed") tiles, batch-pair packing.
  Each pair p handles 8 batches = 2 groups (A=batches 8p..8p+3 in
  partitions 0..63, B=batches 8p+4..8p+7 in partitions 64..127).
  N = 4 batches * 100 positions = 400 columns per group.
"""

import numpy as np

B, S, E = 8192, 100, 64
H1, H2 = 128, 64
NCORES = 8
BC = B // NCORES          # 1024 batches per core
TB = 4                    # batches per group
N = TB * S                # 400 columns per matmul
NP = BC // (2 * TB)       # 128 pairs per core

F32 = None  # set lazily (mybir.dt.float32)

_PROG = {}


def _build_program():
    import concourse.bass as bass
    import concourse.bacc as bacc
    import concourse.tile as tile
    import concourse.mybir as mybir

    f32 = mybir.dt.float32
    nc = bacc.Bacc(
        "TRN2", target_bir_lowering=False, debug=False, num_devices=NCORES)

    # ---- DRAM I/O ----
    # consts packed [128, 515]: wq2|wk2|wc2 (128 each), w1 (64), w2r2 (64),
    # b0 | b1 | b2 (1 col each)
    NCONST = 3 * H1 + 2 * H2 + 3
    kT_d = nc.declare_dram_parameter("kT", [E, BC * S], f32, isOutput=False)
    ql_d = nc.declare_dram_parameter("ql", [NP, 128, 2 * TB], f32, isOutput=False)
    cst_d = nc.declare_dram_parameter("consts", [128, NCONST], f32, isOutput=False)

    attn_d = nc.declare_dram_parameter("attn2", [2 * NP, N], f32, isOutput=True)
    outT_d = nc.declare_dram_parameter("outT", [128, BC // 2], f32, isOutput=True)

    AF = mybir.ActivationFunctionType
    OP = mybir.AluOpType

    with tile.TileContext(nc) as tc:
        with (
            tc.tile_pool(name="consts", bufs=1) as cpool,
            tc.tile_pool(name="kin", bufs=3) as kpool,
            tc.tile_pool(name="qlp", bufs=3) as qlpool,
            tc.tile_pool(name="mid", bufs=2) as mpool,
            tc.tile_pool(name="small", bufs=3) as spool,
            tc.tile_pool(name="psl0", bufs=2, space="PSUM") as psl0pool,
            tc.tile_pool(name="psh2", bufs=2, space="PSUM") as psh2pool,
            tc.tile_pool(name="pss", bufs=2, space="PSUM") as psspool,
            tc.tile_pool(name="acc", bufs=1) as apool,
        ):
            # ---- constants in SBUF (single DMA) ----
            cst_t = cpool.tile([128, NCONST], f32, tag="consts")
            iota_t = cpool.tile([128, N], f32, tag="iota")
            nc.sync.dma_start(cst_t[:], cst_d[:])
            wq_t = cst_t[:, 0:H1]
            wk_t = cst_t[:, H1:2 * H1]
            wc_t = cst_t[:, 2 * H1:3 * H1]
            w1_t = cst_t[:, 3 * H1:3 * H1 + H2]
            w2_t = cst_t[:, 3 * H1 + H2:3 * H1 + 2 * H2]
            b0_t = cst_t[:, 3 * H1 + 2 * H2:3 * H1 + 2 * H2 + 1]
            b1_t = cst_t[:, 3 * H1 + 2 * H2 + 1:3 * H1 + 2 * H2 + 2]
            b2_t = cst_t[:, 3 * H1 + 2 * H2 + 2:3 * H1 + 2 * H2 + 3]
            nc.gpsimd.iota(
                iota_t[:],
                pattern=[[0, TB], [1, S]],
                base=0,
                channel_multiplier=0,
                allow_small_or_imprecise_dtypes=True,
            )

            outT_t = apool.tile([128, BC // 2], f32, tag="outT")

            for p in range(NP):
                gA, gB = 2 * p, 2 * p + 1

                # ---- load keys (feature-major), build q*k ----
                kt = kpool.tile([128, N], f32, tag="kt")
                nc.sync.dma_start(kt[0:E, :], kT_d[:, gA * N:(gA + 1) * N])
                nc.sync.dma_start(kt[E:128, :], kT_d[:, gB * N:(gB + 1) * N])
                ql_t = qlpool.tile([128, 2 * TB], f32, tag="ql")
                nc.sync.dma_start(ql_t[:], ql_d[p])

                q_bc = ql_t[:, 0:TB].unsqueeze(2).broadcast_to((128, TB, S))
                qk = kpool.tile([128, N], f32, tag="qk")
                nc.vector.tensor_tensor(
                    qk[:].rearrange("p (b s) -> p b s", s=S),
                    kt[:].rearrange("p (b s) -> p b s", s=S),
                    q_bc,
                    op=OP.mult,
                )

                # ---- layer 0: h1 = relu(Wk.T k + Wc.T qk + Wq.T q + b0) ----
                # one 2-bank PSUM tile; group A in cols 0:400, B in cols 512:912
                ps0 = psl0pool.tile([128, 1024], f32, tag="ps0")
                qa_bc = ql_t[0:E, 0:TB].unsqueeze(2).broadcast_to((E, TB, S))
                qb_bc = ql_t[E:128, 0:TB].unsqueeze(2).broadcast_to((E, TB, S))
                nc.tensor.matmul(ps0[:, 0:N], wk_t[0:E, :], kt[0:E, :],
                                 start=True, stop=False)
                nc.tensor.matmul(ps0[:, 0:N], wc_t[0:E, :], qk[0:E, :],
                                 start=False, stop=False)
                nc.tensor.matmul(
                    ps0[:, 0:N].rearrange("p (b s) -> p b s", s=S),
                    wq_t[0:E, :], qa_bc, start=False, stop=True)
                nc.tensor.matmul(ps0[:, 512:512 + N], wk_t[E:128, :], kt[E:128, :],
                                 start=True, stop=False)
                nc.tensor.matmul(ps0[:, 512:512 + N], wc_t[E:128, :], qk[E:128, :],
                                 start=False, stop=False)
                nc.tensor.matmul(
                    ps0[:, 512:512 + N].rearrange("p (b s) -> p b s", s=S),
                    wq_t[E:128, :], qb_bc, start=False, stop=True)

                # relu over both groups in one ACT pass (strided PSUM read)
                h1 = mpool.tile([128, 2 * N], f32, tag="h1")
                nc.scalar.activation(
                    h1[:].rearrange("p (c n) -> p c n", c=2),
                    ps0[:].rearrange("p (c n) -> p c n", c=2)[:, :, 0:N],
                    AF.Relu,
                    bias=b0_t[:, 0:1],
                )

                # ---- layer 1: h2 = relu(W1.T h1 + b1) (pair-packed out) ----
                ps1 = psh2pool.tile([128, 512], f32, tag="ps1")
                nc.tensor.matmul(ps1[0:H2, 0:N], w1_t[:], h1[:, 0:N],
                                 start=True, stop=True)
                nc.tensor.matmul(ps1[H2:128, 0:N], w1_t[:], h1[:, N:2 * N],
                                 start=True, stop=True, tile_position=(0, 64))
                h2 = mpool.tile([128, N], f32, tag="h2")
                nc.vector.tensor_scalar(
                    h2[:], ps1[:, 0:N], scalar1=b1_t[:, 0:1], scalar2=0.0,
                    op0=OP.add, op1=OP.max)

                # ---- layer 2: scores (pair-packed, broadcast to 64 partitions) ----
                ps2 = psspool.tile([128, 512], f32, tag="ps2")
                nc.tensor.matmul(ps2[0:H2, 0:N], w2_t[0:H2, :], h2[0:H2, :],
                                 start=True, stop=True)
                nc.tensor.matmul(ps2[H2:128, 0:N], w2_t[H2:128, :], h2[H2:128, :],
                                 start=True, stop=True, tile_position=(64, 64))

                # ---- softmax over s within each batch ----
                expm = mpool.tile([128, N], f32, tag="expm")
                nc.scalar.activation(expm[:], ps2[:, 0:N], AF.Exp,
                                     bias=b2_t[:, 0:1])
                len_bc = ql_t[:, TB:2 * TB].unsqueeze(2).broadcast_to((128, TB, S))
                mask = mpool.tile([128, N], f32, tag="mask")
                nc.vector.tensor_tensor(
                    mask[:].rearrange("p (b s) -> p b s", s=S),
                    iota_t[:].rearrange("p (b s) -> p b s", s=S),
                    len_bc, op=OP.is_lt)
                expm_m = mpool.tile([128, N], f32, tag="expm_m")
                nc.vector.tensor_tensor(expm_m[:], expm[:], mask[:], op=OP.mult)
                sums = spool.tile([128, TB], f32, tag="sums")
                nc.vector.tensor_reduce(
                    sums[:], expm_m[:].rearrange("p (b s) -> p b s", s=S),
                    axis=mybir.AxisListType.X, op=OP.add)
                rec = spool.tile([128, TB], f32, tag="rec")
                nc.vector.reciprocal(rec[:], sums[:])
                attn_t = mpool.tile([128, N], f32, tag="attn")
                nc.vector.tensor_tensor(
                    attn_t[:].rearrange("p (b s) -> p b s", s=S),
                    expm_m[:].rearrange("p (b s) -> p b s", s=S),
                    rec[:].unsqueeze(2).broadcast_to((128, TB, S)),
                    op=OP.mult)
                nc.sync.dma_start(attn_d[gA:gA + 1, :], attn_t[0:1, :])
                nc.sync.dma_start(attn_d[gB:gB + 1, :], attn_t[E:E + 1, :])

                # ---- out = sum_s attn * k  (multiply + segmented reduce) ----
                outw = mpool.tile([128, N], f32, tag="outw")
                nc.vector.tensor_tensor(outw[:], kt[:], attn_t[:], op=OP.mult)
                nc.vector.tensor_reduce(
                    outT_t[:, TB * p:TB * (p + 1)],
                    outw[:].rearrange("p (b s) -> p b s", s=S),
                    axis=mybir.AxisListType.X, op=OP.add)

            nc.sync.dma_start(outT_d[:], outT_t[:])

    nc.compile()
    return nc


def _get_program():
    if "nc" not in _PROG:
        _PROG["nc"] = _build_program()
    return _PROG["nc"]


def kernel(query, keys, keys_length, W0, b0, W1, b1, W2, b2):
    from concourse.bass_utils import run_bass_kernel_spmd

    query = np.asarray(query, dtype=np.float32)
    keys = np.asarray(keys, dtype=np.float32)
    keys_length = np.asarray(keys_length)
    W0 = np.asarray(W0, dtype=np.float32)
    b0 = np.asarray(b0, dtype=np.float32)
    W1 = np.asarray(W1, dtype=np.float32)
    b1 = np.asarray(b1, dtype=np.float32)
    W2 = np.asarray(W2, dtype=np.float32)
    b2 = np.asarray(b2, dtype=np.float32)

    # ---- host-side weight folding (exact algebra) ----
    Wq = W0[0:E] + W0[3 * E:4 * E]          # q coefficient  [64,128]
    Wkk = W0[E:2 * E] - W0[3 * E:4 * E]     # k coefficient  [64,128]
    Wc = W0[2 * E:3 * E]                    # q*k coefficient [64,128]
    wq2 = np.ascontiguousarray(np.concatenate([Wq, Wq], 0))
    wk2 = np.ascontiguousarray(np.concatenate([Wkk, Wkk], 0))
    wc2 = np.ascontiguousarray(np.concatenate([Wc, Wc], 0))
    w2r = np.repeat(W2, H2, axis=1)         # [64,64] replicated cols
    w2r2 = np.ascontiguousarray(np.concatenate([w2r, w2r], 0))
    b0c = b0.reshape(H1, 1)
    b1c = np.concatenate([b1, b1]).reshape(128, 1)
    b2c = np.full((128, 1), float(b2.reshape(-1)[0]), dtype=np.float32)

    nc = _get_program()

    in_maps = []
    for c in range(NCORES):
        kc = keys[c * BC:(c + 1) * BC]                       # [1024,100,64]
        kT = np.ascontiguousarray(kc.transpose(2, 0, 1).reshape(E, BC * S))
        qc = query[c * BC:(c + 1) * BC]                      # [1024,64]
        lc = keys_length[c * BC:(c + 1) * BC].astype(np.float32)
        qpart = qc.reshape(NP, 2, TB, E).transpose(0, 1, 3, 2).reshape(NP, 128, TB)
        lpart = np.broadcast_to(
            lc.reshape(NP, 2, 1, TB), (NP, 2, E, TB)).reshape(NP, 128, TB)
        ql = np.ascontiguousarray(
            np.concatenate([qpart, lpart], axis=2).astype(np.float32))
        w1p = np.zeros((128, H2), dtype=np.float32)
        w1p[:] = W1
        consts = np.ascontiguousarray(np.concatenate(
            [wq2, wk2, wc2, w1p, w2r2, b0c, b1c, b2c], axis=1))
        in_maps.append({"kT": kT, "ql": ql, "consts": consts})

    bkr = run_bass_kernel_spmd(nc, in_maps, list(range(NCORES)))
    _PROG["last_results"] = bkr
    res = bkr.results

    out = np.empty((B, E), dtype=np.float32)
    attn = np.empty((B, S), dtype=np.float32)
    for c in range(NCORES):
        attn[c * BC:(c + 1) * BC] = res[c]["attn2"].reshape(BC, S)
        oT = res[c]["outT"]                                   # [128, 512]
        # outT[half*64+e, TB*p+cb] -> out[8p + half*4 + cb, e]
        out[c * BC:(c + 1) * BC] = (
            oT.reshape(2, E, NP, TB).transpose(2, 0, 3, 1).reshape(BC, E))
    return out, attn


# revision 15
# speedup vs baseline: 2.3685x; 2.3685x over previous
"""Trainium2 Bass kernel for nn_AttentionLayer (DIN-style attention scoring MLP).

Math (per batch b, key position s):
    feats = [q, k, q*k, q-k]                       # [4E] = 256
    h1 = relu(feats @ W0 + b0)                     # 128
    h2 = relu(h1 @ W1 + b1)                        # 64
    score = h2 @ W2 + b2                           # scalar
    attn = softmax_s(score masked to s < len[b])
    out = sum_s attn[s] * k[s]                     # [E]

Host-side algebra (exact):
    W0 rows: [0:64]=Wq(q), [64:128]=Wkk(k), [128:192]=Wc(q*k), [192:256]=Wd(q-k)
    h1 = relu(q@(Wq+Wd) + k@(Wkk-Wd) + (q*k)@Wc + b0)
  Invalid key columns (s >= len[b]) are zeroed on the host, so the device's
  unnormalized contraction sum_s exp(score)*k skips them exactly; the softmax
  normalization (divide by masked sum) happens on the host during the gather.

Device layout: feature-major ("transposed") tiles, batch-pair packing.
  Each pair p handles 8 batches = 2 groups (A=batches 8p..8p+3 in
  partitions 0..63, B=batches 8p+4..8p+7 in partitions 64..127).
  N = 4 batches * 100 positions = 400 columns per group.
"""

import numpy as np

B, S, E = 8192, 100, 64
H1, H2 = 128, 64
NCORES = 8
BC = B // NCORES          # 1024 batches per core
TB = 4                    # batches per group
N = TB * S                # 400 columns per matmul
NP = BC // (2 * TB)       # 128 pairs per core

MM_DTYPE = "bfloat16"     # "bfloat16" | "float32r" | "float32"

_PROG = {}


def _build_program():
    import concourse.bacc as bacc
    import concourse.tile as tile
    import concourse.mybir as mybir

    f32 = mybir.dt.float32
    dt_mm = getattr(mybir.dt, MM_DTYPE)
    nc = bacc.Bacc(
        "TRN2", target_bir_lowering=False, debug=False, num_devices=NCORES)

    # ---- DRAM I/O ----
    # kTP pair-packed: [128, NP*N]; rows 0:64 = keys^T of group 2p,
    # rows 64:128 = keys^T of group 2p+1, at columns p*N:(p+1)*N.
    kT_d = nc.declare_dram_parameter("kTP", [128, NP * N], dt_mm, isOutput=False)
    ql_d = nc.declare_dram_parameter("qlP", [NP, 128, TB], dt_mm, isOutput=False)
    # weights packed [128, 3*H1 + 2*H2] in matmul dtype
    NW = 3 * H1 + 2 * H2
    w_d = nc.declare_dram_parameter("wpack", [128, NW], dt_mm, isOutput=False)
    b_d = nc.declare_dram_parameter("bpack", [128, 3], f32, isOutput=False)

    em_d = nc.declare_dram_parameter("em2", [2 * NP, N], f32, isOutput=True)
    outT_d = nc.declare_dram_parameter("outT", [128, BC // 2], f32, isOutput=True)

    AF = mybir.ActivationFunctionType
    OP = mybir.AluOpType

    with tile.TileContext(nc) as tc:
        with (
            tc.tile_pool(name="consts", bufs=1) as cpool,
            tc.tile_pool(name="kin", bufs=4) as kpool,
            tc.tile_pool(name="qlp", bufs=4) as qlpool,
            tc.tile_pool(name="mid", bufs=3) as mpool,
            tc.tile_pool(name="psl0", bufs=2, space="PSUM") as psl0pool,
            tc.tile_pool(name="psh2", bufs=2, space="PSUM") as psh2pool,
            tc.tile_pool(name="pss", bufs=2, space="PSUM") as psspool,
            tc.tile_pool(name="acc", bufs=1) as apool,
        ):
            w_t = cpool.tile([128, NW], dt_mm, tag="wpack")
            b_t = cpool.tile([128, 3], f32, tag="bpack")
            nc.sync.dma_start(w_t[:], w_d[:])
            nc.sync.dma_start(b_t[:], b_d[:])
            wq_t = w_t[:, 0:H1]
            wk_t = w_t[:, H1:2 * H1]
            wc_t = w_t[:, 2 * H1:3 * H1]
            w1_t = w_t[:, 3 * H1:3 * H1 + H2]
            w2_t = w_t[:, 3 * H1 + H2:3 * H1 + 2 * H2]
            b0_t = b_t[:, 0:1]
            b1_t = b_t[:, 1:2]
            b2_t = b_t[:, 2:3]

            outT_t = apool.tile([128, BC // 2], f32, tag="outT")

            for p in range(NP):
                # ---- load keys (feature-major, pair-packed), build q*k ----
                kt = kpool.tile([128, N], dt_mm, tag="kt")
                nc.sync.dma_start(kt[:], kT_d[:, p * N:(p + 1) * N])
                ql_t = qlpool.tile([128, TB], dt_mm, tag="ql")
                nc.sync.dma_start(ql_t[:], ql_d[p])

                q_bc = ql_t[:].unsqueeze(2).broadcast_to((128, TB, S))
                qk = kpool.tile([128, N], dt_mm, tag="qk")
                nc.vector.tensor_tensor(
                    qk[:].rearrange("p (b s) -> p b s", s=S),
                    kt[:].rearrange("p (b s) -> p b s", s=S),
                    q_bc,
                    op=OP.mult,
                )

                # ---- layer 0: h1 = relu(Wk.T k + Wc.T qk + Wq.T q + b0) ----
                ps0 = psl0pool.tile([128, 1024], f32, tag="ps0")
                qa_bc = ql_t[0:E, :].unsqueeze(2).broadcast_to((E, TB, S))
                qb_bc = ql_t[E:128, :].unsqueeze(2).broadcast_to((E, TB, S))
                nc.tensor.matmul(ps0[:, 0:N], wk_t[0:E, :], kt[0:E, :],
                                 start=True, stop=False)
                nc.tensor.matmul(ps0[:, 0:N], wc_t[0:E, :], qk[0:E, :],
                                 start=False, stop=False)
                nc.tensor.matmul(
                    ps0[:, 0:N].rearrange("p (b s) -> p b s", s=S),
                    wq_t[0:E, :], qa_bc, start=False, stop=True)
                nc.tensor.matmul(ps0[:, 512:512 + N], wk_t[E:128, :], kt[E:128, :],
                                 start=True, stop=False)
                nc.tensor.matmul(ps0[:, 512:512 + N], wc_t[E:128, :], qk[E:128, :],
                                 start=False, stop=False)
                nc.tensor.matmul(
                    ps0[:, 512:512 + N].rearrange("p (b s) -> p b s", s=S),
                    wq_t[E:128, :], qb_bc, start=False, stop=True)

                # relu over both groups in one ACT pass (strided PSUM read)
                h1 = mpool.tile([128, 2 * N], dt_mm, tag="h1")
                nc.scalar.activation(
                    h1[:].rearrange("p (c n) -> p c n", c=2),
                    ps0[:].rearrange("p (c n) -> p c n", c=2)[:, :, 0:N],
                    AF.Relu,
                    bias=b0_t,
                )

                # ---- layer 1: h2 = relu(W1.T h1 + b1) (pair-packed out) ----
                ps1 = psh2pool.tile([128, 512], f32, tag="ps1")
                nc.tensor.matmul(ps1[0:H2, 0:N], w1_t[:], h1[:, 0:N],
                                 start=True, stop=True)
                nc.tensor.matmul(ps1[H2:128, 0:N], w1_t[:], h1[:, N:2 * N],
                                 start=True, stop=True, tile_position=(0, 64))
                h2 = mpool.tile([128, N], dt_mm, tag="h2")
                nc.vector.tensor_scalar(
                    h2[:], ps1[:, 0:N], scalar1=b1_t, scalar2=0.0,
                    op0=OP.add, op1=OP.max)

                # ---- layer 2: scores (pair-packed, broadcast over 64 parts) ----
                ps2 = psspool.tile([128, 512], f32, tag="ps2")
                nc.tensor.matmul(ps2[0:H2, 0:N], w2_t[0:H2, :], h2[0:H2, :],
                                 start=True, stop=True)
                nc.tensor.matmul(ps2[H2:128, 0:N], w2_t[H2:128, :], h2[H2:128, :],
                                 start=True, stop=True, tile_position=(64, 64))

                # ---- exp (unnormalized softmax numerator) ----
                expm = mpool.tile([128, N], f32, tag="expm")
                nc.scalar.activation(expm[:], ps2[:, 0:N], AF.Exp, bias=b2_t)
                nc.sync.dma_start(em_d[2 * p:2 * p + 1, :], expm[0:1, :])
                nc.sync.dma_start(em_d[2 * p + 1:2 * p + 2, :], expm[E:E + 1, :])

                # ---- unnormalized out = sum_s expm * k (invalid k cols are 0) --
                outw = mpool.tile([128, N], f32, tag="outw")
                nc.vector.tensor_tensor(outw[:], kt[:], expm[:], op=OP.mult)
                nc.vector.tensor_reduce(
                    outT_t[:, TB * p:TB * (p + 1)],
                    outw[:].rearrange("p (b s) -> p b s", s=S),
                    axis=mybir.AxisListType.X, op=OP.add)

            nc.sync.dma_start(outT_d[:], outT_t[:])

    nc.compile()
    return nc


def _get_program():
    if "nc" not in _PROG:
        _PROG["nc"] = _build_program()
    return _PROG["nc"]


def _np_mm_dtype():
    if MM_DTYPE == "bfloat16":
        import ml_dtypes
        return np.dtype(ml_dtypes.bfloat16)
    return np.dtype(np.float32)


def kernel(query, keys, keys_length, W0, b0, W1, b1, W2, b2):
    from concourse.bass_utils import run_bass_kernel_spmd

    query = np.asarray(query, dtype=np.float32)
    keys = np.asarray(keys, dtype=np.float32)
    keys_length = np.asarray(keys_length)
    W0 = np.asarray(W0, dtype=np.float32)
    b0 = np.asarray(b0, dtype=np.float32)
    W1 = np.asarray(W1, dtype=np.float32)
    b1 = np.asarray(b1, dtype=np.float32)
    W2 = np.asarray(W2, dtype=np.float32)
    b2 = np.asarray(b2, dtype=np.float32)
    npdt = _np_mm_dtype()

    # ---- host-side weight folding (exact algebra) ----
    Wq = W0[0:E] + W0[3 * E:4 * E]
    Wkk = W0[E:2 * E] - W0[3 * E:4 * E]
    Wc = W0[2 * E:3 * E]
    wq2 = np.concatenate([Wq, Wq], 0)
    wk2 = np.concatenate([Wkk, Wkk], 0)
    wc2 = np.concatenate([Wc, Wc], 0)
    w1p = np.zeros((128, H2), np.float32)
    w1p[:] = W1
    w2r = np.repeat(W2, H2, axis=1)
    w2r2 = np.concatenate([w2r, w2r], 0)
    wpack = np.ascontiguousarray(np.concatenate(
        [wq2, wk2, wc2, w1p, w2r2], axis=1).astype(npdt))
    bpack = np.zeros((128, 3), np.float32)
    bpack[:, 0] = b0
    bpack[:, 1] = np.concatenate([b1, b1])
    bpack[:, 2] = float(b2.reshape(-1)[0])

    mask_full = (np.arange(S)[None, :] < keys_length[:, None])      # [B,S]

    nc = _get_program()

    in_maps = []
    for c in range(NCORES):
        kc = keys[c * BC:(c + 1) * BC] * mask_full[c * BC:(c + 1) * BC, :, None]
        # [1024,100,64] -> feature-major pair-packed [128, NP*400]
        kt = kc.transpose(2, 0, 1).reshape(E, BC * S)               # [64, 102400]
        kTP = np.ascontiguousarray(
            kt.reshape(E, NP, 2, N).transpose(2, 0, 1, 3).reshape(128, NP * N)
        ).astype(npdt)
        qc = query[c * BC:(c + 1) * BC]
        qlP = np.ascontiguousarray(
            qc.reshape(NP, 2, TB, E).transpose(0, 1, 3, 2).reshape(NP, 128, TB)
        ).astype(npdt)
        in_maps.append({"kTP": kTP, "qlP": qlP, "wpack": wpack, "bpack": bpack})

    bkr = run_bass_kernel_spmd(nc, in_maps, list(range(NCORES)))
    _PROG["last_results"] = bkr
    res = bkr.results

    out = np.empty((B, E), dtype=np.float32)
    attn = np.empty((B, S), dtype=np.float32)
    for c in range(NCORES):
        em = res[c]["em2"].reshape(BC, S).astype(np.float64)
        m = mask_full[c * BC:(c + 1) * BC]
        em = em * m
        sums = em.sum(1, keepdims=True)                              # [BC,1]
        attn[c * BC:(c + 1) * BC] = (em / sums).astype(np.float32)
        oT = res[c]["outT"]                                          # [128, 512]
        o = oT.reshape(2, E, NP, TB).transpose(2, 0, 3, 1).reshape(BC, E)
        out[c * BC:(c + 1) * BC] = (o / sums).astype(np.float32)
    return out, attn


# revision 21
# speedup vs baseline: 3.0587x; 1.2914x over previous
"""Trainium2 Bass kernel for nn_AttentionLayer (DIN-style attention scoring MLP).

Math (per batch b, key position s):
    feats = [q, k, q*k, q-k]                       # [4E] = 256
    h1 = relu(feats @ W0 + b0)                     # 128
    h2 = relu(h1 @ W1 + b1)                        # 64
    score = h2 @ W2 + b2                           # scalar
    attn = softmax_s(score masked to s < len[b])
    out = sum_s attn[s] * k[s]                     # [E]

Host-side algebra (exact):
    W0 rows: [0:64]=Wq(q), [64:128]=Wkk(k), [128:192]=Wc(q*k), [192:256]=Wd(q-k)
    h1 = relu(q@(Wq+Wd) + k@(Wkk-Wd) + (q*k)@Wc + b0)
  Invalid key columns (s >= len[b]) are zeroed on the host, so the device's
  unnormalized contraction sum_s exp(score)*k skips them exactly; the softmax
  normalization (divide by masked sum) happens on the host during the gather.

Device layout: feature-major ("transposed") tiles, batch-pair packing.
  Each pair p handles 8 batches = 2 groups (A=batches 8p..8p+3 in
  partitions 0..63, B=batches 8p+4..8p+7 in partitions 64..127).
  N = 4 batches * 100 positions = 400 columns per group.
"""

import numpy as np

B, S, E = 8192, 100, 64
H1, H2 = 128, 64
NCORES = 8
BC = B // NCORES          # 1024 batches per core
TB = 4                    # batches per group
N = TB * S                # 400 columns per matmul
NP = BC // (2 * TB)       # 128 pairs per core

MM_DTYPE = "bfloat16"     # "bfloat16" | "float32r" | "float32"

_PROG = {}


def _build_program():
    import concourse.bacc as bacc
    import concourse.tile as tile
    import concourse.mybir as mybir

    f32 = mybir.dt.float32
    dt_mm = getattr(mybir.dt, MM_DTYPE)
    nc = bacc.Bacc(
        "TRN2", target_bir_lowering=False, debug=False, num_devices=NCORES)

    # ---- DRAM I/O ----
    # kTP pair-packed: [128, NP*N]; rows 0:64 = keys^T of group 2p,
    # rows 64:128 = keys^T of group 2p+1, at columns p*N:(p+1)*N.
    kT_d = nc.declare_dram_parameter("kTP", [128, NP * N], dt_mm, isOutput=False)
    ql_d = nc.declare_dram_parameter("qlP", [NP, 128, TB], dt_mm, isOutput=False)
    # weights packed [128, 3*H1 + 2*H2] in matmul dtype
    NW = 3 * H1 + 2 * H2
    w_d = nc.declare_dram_parameter("wpack", [128, NW], dt_mm, isOutput=False)
    b_d = nc.declare_dram_parameter("bpack", [128, 3], f32, isOutput=False)

    em_d = nc.declare_dram_parameter("em2", [2 * NP, N], f32, isOutput=True)
    outT_d = nc.declare_dram_parameter("outT", [128, BC // 2], f32, isOutput=True)

    AF = mybir.ActivationFunctionType
    OP = mybir.AluOpType

    with tile.TileContext(nc) as tc:
        with (
            tc.tile_pool(name="consts", bufs=1) as cpool,
            tc.tile_pool(name="kin", bufs=4) as kpool,
            tc.tile_pool(name="qlp", bufs=4) as qlpool,
            tc.tile_pool(name="mid", bufs=3) as mpool,
            tc.tile_pool(name="psl0", bufs=2, space="PSUM") as psl0pool,
            tc.tile_pool(name="psh2", bufs=2, space="PSUM") as psh2pool,
            tc.tile_pool(name="pss", bufs=2, space="PSUM") as psspool,
            tc.tile_pool(name="acc", bufs=1) as apool,
        ):
            w_t = cpool.tile([128, NW], dt_mm, tag="wpack")
            b_t = cpool.tile([128, 3], f32, tag="bpack")
            nc.sync.dma_start(w_t[:], w_d[:])
            nc.sync.dma_start(b_t[:], b_d[:])
            wq_t = w_t[:, 0:H1]
            wk_t = w_t[:, H1:2 * H1]
            wc_t = w_t[:, 2 * H1:3 * H1]
            w1_t = w_t[:, 3 * H1:3 * H1 + H2]
            w2_t = w_t[:, 3 * H1 + H2:3 * H1 + 2 * H2]
            b0_t = b_t[:, 0:1]
            b1_t = b_t[:, 1:2]
            b2_t = b_t[:, 2:3]

            outT_t = apool.tile([128, BC // 2], f32, tag="outT")

            KB = min(4, NP)    # pairs per keys-block DMA / qk op
            QB = min(8, NP)    # pairs per ql block DMA
            ktb = None
            qkb = None
            qlb = None
            for p in range(NP):
                # ---- block loads: keys (4 pairs), q (8 pairs) ----
                if p % QB == 0:
                    qlb = qlpool.tile([128, QB * TB], dt_mm, tag="ql")
                    nc.sync.dma_start(
                        qlb[:].rearrange("p (q t) -> p q t", q=QB),
                        ql_d[p:p + QB].rearrange("q p t -> p q t"))
                if p % KB == 0:
                    ktb = kpool.tile([128, KB * N], dt_mm, tag="kt")
                    nc.sync.dma_start(ktb[:], kT_d[:, p * N:(p + KB) * N])
                    qkb = kpool.tile([128, KB * N], dt_mm, tag="qk")
                    q_bc = (qlb[:, (p % QB) * TB:(p % QB + KB) * TB]
                            .unsqueeze(2).broadcast_to((128, KB * TB, S)))
                    nc.vector.tensor_tensor(
                        qkb[:].rearrange("p (b s) -> p b s", s=S),
                        ktb[:].rearrange("p (b s) -> p b s", s=S),
                        q_bc,
                        op=OP.mult,
                    )
                kt = ktb[:, (p % KB) * N:(p % KB + 1) * N]
                qk = qkb[:, (p % KB) * N:(p % KB + 1) * N]
                ql_t = qlb[:, (p % QB) * TB:(p % QB + 1) * TB]

                # ---- layer 0: h1 = relu(Wk.T k + Wc.T qk + Wq.T q + b0) ----
                ps0 = psl0pool.tile([128, 1024], f32, tag="ps0")
                qa_bc = ql_t[0:E].unsqueeze(2).broadcast_to((E, TB, S))
                qb_bc = ql_t[E:128].unsqueeze(2).broadcast_to((E, TB, S))
                nc.tensor.matmul(ps0[:, 0:N], wk_t[0:E, :], kt[0:E],
                                 start=True, stop=False)
                nc.tensor.matmul(ps0[:, 0:N], wc_t[0:E, :], qk[0:E],
                                 start=False, stop=False)
                nc.tensor.matmul(
                    ps0[:, 0:N].rearrange("p (b s) -> p b s", s=S),
                    wq_t[0:E, :], qa_bc, start=False, stop=True)
                nc.tensor.matmul(ps0[:, 512:512 + N], wk_t[E:128, :], kt[E:128],
                                 start=True, stop=False)
                nc.tensor.matmul(ps0[:, 512:512 + N], wc_t[E:128, :], qk[E:128],
                                 start=False, stop=False)
                nc.tensor.matmul(
                    ps0[:, 512:512 + N].rearrange("p (b s) -> p b s", s=S),
                    wq_t[E:128, :], qb_bc, start=False, stop=True)

                # relu over both groups in one ACT pass (strided PSUM read)
                h1 = mpool.tile([128, 2 * N], dt_mm, tag="h1")
                nc.scalar.activation(
                    h1[:].rearrange("p (c n) -> p c n", c=2),
                    ps0[:].rearrange("p (c n) -> p c n", c=2)[:, :, 0:N],
                    AF.Relu,
                    bias=b0_t,
                )

                # ---- layer 1: h2 = relu(W1.T h1 + b1) (pair-packed out) ----
                ps1 = psh2pool.tile([128, 512], f32, tag="ps1")
                nc.tensor.matmul(ps1[0:H2, 0:N], w1_t[:], h1[:, 0:N],
                                 start=True, stop=True)
                nc.tensor.matmul(ps1[H2:128, 0:N], w1_t[:], h1[:, N:2 * N],
                                 start=True, stop=True, tile_position=(0, 64))
                h2 = mpool.tile([128, N], dt_mm, tag="h2")
                nc.vector.tensor_scalar(
                    h2[:], ps1[:, 0:N], scalar1=b1_t, scalar2=0.0,
                    op0=OP.add, op1=OP.max)

                # ---- layer 2: scores (pair-packed, broadcast over 64 parts) ----
                ps2 = psspool.tile([128, 512], f32, tag="ps2")
                nc.tensor.matmul(ps2[0:H2, 0:N], w2_t[0:H2, :], h2[0:H2, :],
                                 start=True, stop=True)
                nc.tensor.matmul(ps2[H2:128, 0:N], w2_t[H2:128, :], h2[H2:128, :],
                                 start=True, stop=True, tile_position=(64, 64))

                # ---- exp (unnormalized softmax numerator) ----
                expm = mpool.tile([128, N], f32, tag="expm")
                nc.scalar.activation(expm[:], ps2[:, 0:N], AF.Exp, bias=b2_t)
                nc.sync.dma_start(em_d[2 * p:2 * p + 2, :],
                                  expm[0:E + 1:E, :])

                # ---- unnormalized out = sum_s expm * k (invalid k cols are 0) --
                outw = mpool.tile([128, N], f32, tag="outw")
                nc.vector.tensor_tensor(outw[:], kt, expm[:], op=OP.mult)
                nc.vector.tensor_reduce(
                    outT_t[:, TB * p:TB * (p + 1)],
                    outw[:].rearrange("p (b s) -> p b s", s=S),
                    axis=mybir.AxisListType.X, op=OP.add)

            nc.sync.dma_start(outT_d[:], outT_t[:])

    nc.compile()
    return nc


def _get_program():
    if "nc" not in _PROG:
        _PROG["nc"] = _build_program()
    return _PROG["nc"]


def _np_mm_dtype():
    if MM_DTYPE == "bfloat16":
        import ml_dtypes
        return np.dtype(ml_dtypes.bfloat16)
    return np.dtype(np.float32)


def kernel(query, keys, keys_length, W0, b0, W1, b1, W2, b2):
    from concourse.bass_utils import run_bass_kernel_spmd

    query = np.asarray(query, dtype=np.float32)
    keys = np.asarray(keys, dtype=np.float32)
    keys_length = np.asarray(keys_length)
    W0 = np.asarray(W0, dtype=np.float32)
    b0 = np.asarray(b0, dtype=np.float32)
    W1 = np.asarray(W1, dtype=np.float32)
    b1 = np.asarray(b1, dtype=np.float32)
    W2 = np.asarray(W2, dtype=np.float32)
    b2 = np.asarray(b2, dtype=np.float32)
    npdt = _np_mm_dtype()

    # ---- host-side weight folding (exact algebra) ----
    Wq = W0[0:E] + W0[3 * E:4 * E]
    Wkk = W0[E:2 * E] - W0[3 * E:4 * E]
    Wc = W0[2 * E:3 * E]
    wq2 = np.concatenate([Wq, Wq], 0)
    wk2 = np.concatenate([Wkk, Wkk], 0)
    wc2 = np.concatenate([Wc, Wc], 0)
    w1p = np.zeros((128, H2), np.float32)
    w1p[:] = W1
    w2r = np.repeat(W2, H2, axis=1)
    w2r2 = np.concatenate([w2r, w2r], 0)
    wpack = np.ascontiguousarray(np.concatenate(
        [wq2, wk2, wc2, w1p, w2r2], axis=1).astype(npdt))
    bpack = np.zeros((128, 3), np.float32)
    bpack[:, 0] = b0
    bpack[:, 1] = np.concatenate([b1, b1])
    bpack[:, 2] = float(b2.reshape(-1)[0])

    mask_full = (np.arange(S)[None, :] < keys_length[:, None])      # [B,S]

    nc = _get_program()

    in_maps = []
    for c in range(NCORES):
        kc = keys[c * BC:(c + 1) * BC] * mask_full[c * BC:(c + 1) * BC, :, None]
        # [1024,100,64] -> feature-major pair-packed [128, NP*400]
        kt = kc.transpose(2, 0, 1).reshape(E, BC * S)               # [64, 102400]
        kTP = np.ascontiguousarray(
            kt.reshape(E, NP, 2, N).transpose(2, 0, 1, 3).reshape(128, NP * N)
        ).astype(npdt)
        qc = query[c * BC:(c + 1) * BC]
        qlP = np.ascontiguousarray(
            qc.reshape(NP, 2, TB, E).transpose(0, 1, 3, 2).reshape(NP, 128, TB)
        ).astype(npdt)
        in_maps.append({"kTP": kTP, "qlP": qlP, "wpack": wpack, "bpack": bpack})

    bkr = run_bass_kernel_spmd(nc, in_maps, list(range(NCORES)))
    _PROG["last_results"] = bkr
    res = bkr.results

    out = np.empty((B, E), dtype=np.float32)
    attn = np.empty((B, S), dtype=np.float32)
    for c in range(NCORES):
        em = res[c]["em2"].reshape(BC, S).astype(np.float64)
        m = mask_full[c * BC:(c + 1) * BC]
        em = em * m
        sums = em.sum(1, keepdims=True)                              # [BC,1]
        attn[c * BC:(c + 1) * BC] = (em / sums).astype(np.float32)
        oT = res[c]["outT"]                                          # [128, 512]
        o = oT.reshape(2, E, NP, TB).transpose(2, 0, 3, 1).reshape(BC, E)
        out[c * BC:(c + 1) * BC] = (o / sums).astype(np.float32)
    return out, attn


# revision 28
# speedup vs baseline: 3.3202x; 1.0855x over previous
"""Trainium2 Bass kernel for nn_AttentionLayer (DIN-style attention scoring MLP).

Math (per batch b, key position s):
    feats = [q, k, q*k, q-k]                       # [4E] = 256
    h1 = relu(feats @ W0 + b0)                     # 128
    h2 = relu(h1 @ W1 + b1)                        # 64
    score = h2 @ W2 + b2                           # scalar
    attn = softmax_s(score masked to s < len[b])
    out = sum_s attn[s] * k[s]                     # [E]

Host-side algebra (exact):
    W0 rows: [0:64]=Wq(q), [64:128]=Wkk(k), [128:192]=Wc(q*k), [192:256]=Wd(q-k)
    h1 = relu(q@(Wq+Wd) + k@(Wkk-Wd) + (q*k)@Wc + b0)
  Invalid key columns (s >= len[b]) are zeroed on the host, so the device's
  unnormalized contraction sum_s exp(score)*k skips them exactly; the softmax
  normalization (divide by masked sum) happens on the host during the gather.

Device layout: feature-major ("transposed") tiles, batch-pair packing.
  Each pair p handles 8 batches = 2 groups (A=batches 8p..8p+3 in
  partitions 0..63, B=batches 8p+4..8p+7 in partitions 64..127).
  N = 4 batches * 100 positions = 400 columns per group.
"""

import numpy as np

B, S, E = 8192, 100, 64
H1, H2 = 128, 64
NCORES = 8
BC = B // NCORES          # 1024 batches per core
TB = 4                    # batches per group
N = TB * S                # 400 columns per matmul
NP = BC // (2 * TB)       # 128 pairs per core

MM_DTYPE = "bfloat16"     # "bfloat16" | "float32r" | "float32"

_PROG = {}


def _build_program():
    import concourse.bacc as bacc
    import concourse.tile as tile
    import concourse.mybir as mybir

    f32 = mybir.dt.float32
    dt_mm = getattr(mybir.dt, MM_DTYPE)
    nc = bacc.Bacc(
        "TRN2", target_bir_lowering=False, debug=False, num_devices=NCORES)

    # ---- DRAM I/O ----
    # kTP pair-packed: [128, NP*N]; rows 0:64 = keys^T of group 2p,
    # rows 64:128 = keys^T of group 2p+1, at columns p*N:(p+1)*N.
    kT_d = nc.declare_dram_parameter("kTP", [128, NP * N], dt_mm, isOutput=False)
    kTF_d = nc.declare_dram_parameter("kTF", [128, NP * N], f32, isOutput=False)
    ql_d = nc.declare_dram_parameter("qlP", [NP, 128, TB], dt_mm, isOutput=False)
    # weights packed [128, 3*H1 + 2*H2] in matmul dtype
    NW = 3 * H1 + 2 * H2
    w_d = nc.declare_dram_parameter("wpack", [128, NW], dt_mm, isOutput=False)
    b_d = nc.declare_dram_parameter("bpack", [128, 3], f32, isOutput=False)

    em_d = nc.declare_dram_parameter("em2", [2 * NP, N], f32, isOutput=True)
    outT_d = nc.declare_dram_parameter("outT", [128, BC // 2], f32, isOutput=True)

    AF = mybir.ActivationFunctionType
    OP = mybir.AluOpType

    with tile.TileContext(nc) as tc:
        with (
            tc.tile_pool(name="consts", bufs=1) as cpool,
            tc.tile_pool(name="kin", bufs=4) as kpool,
            tc.tile_pool(name="qlp", bufs=4) as qlpool,
            tc.tile_pool(name="mid", bufs=3) as mpool,
            tc.tile_pool(name="psl0", bufs=2, space="PSUM") as psl0pool,
            tc.tile_pool(name="psh2", bufs=2, space="PSUM") as psh2pool,
            tc.tile_pool(name="pss", bufs=2, space="PSUM") as psspool,
            tc.tile_pool(name="acc", bufs=1) as apool,
        ):
            w_t = cpool.tile([128, NW], dt_mm, tag="wpack")
            b_t = cpool.tile([128, 3], f32, tag="bpack")
            nc.sync.dma_start(w_t[:], w_d[:])
            nc.sync.dma_start(b_t[:], b_d[:])
            wq_t = w_t[:, 0:H1]
            wk_t = w_t[:, H1:2 * H1]
            wc_t = w_t[:, 2 * H1:3 * H1]
            w1_t = w_t[:, 3 * H1:3 * H1 + H2]
            w2_t = w_t[:, 3 * H1 + H2:3 * H1 + 2 * H2]
            b0_t = b_t[:, 0:1]
            b1_t = b_t[:, 1:2]
            b2_t = b_t[:, 2:3]

            outT_t = apool.tile([128, BC // 2], f32, tag="outT")

            KB = min(4, NP)    # pairs per keys-block DMA / qk op
            QB = min(8, NP)    # pairs per ql block DMA
            ktb = None
            qkb = None
            qlb = None
            for p in range(NP):
                # ---- block loads: keys (4 pairs), q (8 pairs) ----
                if p % QB == 0:
                    qlb = qlpool.tile([128, QB * TB], dt_mm, tag="ql")
                    nc.sync.dma_start(
                        qlb[:].rearrange("p (q t) -> p q t", q=QB),
                        ql_d[p:p + QB].rearrange("q p t -> p q t"))
                if p % KB == 0:
                    ktb = kpool.tile([128, KB * N], dt_mm, tag="kt")
                    nc.sync.dma_start(ktb[:], kT_d[:, p * N:(p + KB) * N])
                    ktfb = kpool.tile([128, KB * N], f32, tag="ktf")
                    nc.sync.dma_start(ktfb[:], kTF_d[:, p * N:(p + KB) * N])
                    qkb = kpool.tile([128, KB * N], dt_mm, tag="qk")
                    q_bc = (qlb[:, (p % QB) * TB:(p % QB + KB) * TB]
                            .unsqueeze(2).broadcast_to((128, KB * TB, S)))
                    nc.vector.tensor_tensor(
                        qkb[:].rearrange("p (b s) -> p b s", s=S),
                        ktb[:].rearrange("p (b s) -> p b s", s=S),
                        q_bc,
                        op=OP.mult,
                    )
                kt = ktb[:, (p % KB) * N:(p % KB + 1) * N]
                ktf = ktfb[:, (p % KB) * N:(p % KB + 1) * N]
                qk = qkb[:, (p % KB) * N:(p % KB + 1) * N]
                ql_t = qlb[:, (p % QB) * TB:(p % QB + 1) * TB]

                # ---- layer 0: h1 = relu(Wk.T k + Wc.T qk + Wq.T q + b0) ----
                ps0 = psl0pool.tile([128, 1024], f32, tag="ps0")
                qa_bc = ql_t[0:E].unsqueeze(2).broadcast_to((E, TB, S))
                qb_bc = ql_t[E:128].unsqueeze(2).broadcast_to((E, TB, S))
                nc.tensor.matmul(ps0[:, 0:N], wk_t[0:E, :], kt[0:E],
                                 start=True, stop=False)
                nc.tensor.matmul(ps0[:, 0:N], wc_t[0:E, :], qk[0:E],
                                 start=False, stop=False)
                nc.tensor.matmul(
                    ps0[:, 0:N].rearrange("p (b s) -> p b s", s=S),
                    wq_t[0:E, :], qa_bc, start=False, stop=True)
                nc.tensor.matmul(ps0[:, 512:512 + N], wk_t[E:128, :], kt[E:128],
                                 start=True, stop=False)
                nc.tensor.matmul(ps0[:, 512:512 + N], wc_t[E:128, :], qk[E:128],
                                 start=False, stop=False)
                nc.tensor.matmul(
                    ps0[:, 512:512 + N].rearrange("p (b s) -> p b s", s=S),
                    wq_t[E:128, :], qb_bc, start=False, stop=True)

                # relu over both groups in one ACT pass (strided PSUM read)
                h1 = mpool.tile([128, 2 * N], dt_mm, tag="h1")
                nc.scalar.activation(
                    h1[:].rearrange("p (c n) -> p c n", c=2),
                    ps0[:].rearrange("p (c n) -> p c n", c=2)[:, :, 0:N],
                    AF.Relu,
                    bias=b0_t,
                )

                # ---- layer 1: h2 = relu(W1.T h1 + b1) (pair-packed out) ----
                ps1 = psh2pool.tile([128, 512], f32, tag="ps1")
                nc.tensor.matmul(ps1[0:H2, 0:N], w1_t[:], h1[:, 0:N],
                                 start=True, stop=True)
                nc.tensor.matmul(ps1[H2:128, 0:N], w1_t[:], h1[:, N:2 * N],
                                 start=True, stop=True, tile_position=(0, 64))
                h2 = mpool.tile([128, N], dt_mm, tag="h2")
                if p % 2 == 0:
                    nc.scalar.activation(h2[:], ps1[:, 0:N], AF.Relu, bias=b1_t)
                else:
                    nc.vector.tensor_scalar(
                        h2[:], ps1[:, 0:N], scalar1=b1_t, scalar2=0.0,
                        op0=OP.add, op1=OP.max)

                # ---- layer 2: scores (pair-packed, broadcast over 64 parts) ----
                ps2 = psspool.tile([128, 512], f32, tag="ps2")
                nc.tensor.matmul(ps2[0:H2, 0:N], w2_t[0:H2, :], h2[0:H2, :],
                                 start=True, stop=True)
                nc.tensor.matmul(ps2[H2:128, 0:N], w2_t[H2:128, :], h2[H2:128, :],
                                 start=True, stop=True, tile_position=(64, 64))

                # ---- exp (unnormalized softmax numerator) ----
                expm = mpool.tile([128, N], f32, tag="expm")
                nc.scalar.activation(expm[:], ps2[:, 0:N], AF.Exp, bias=b2_t)
                nc.sync.dma_start(em_d[2 * p:2 * p + 2, :],
                                  expm[0:E + 1:E, :])

                # ---- unnormalized out = sum_s expm * k (invalid k cols are 0) --
                outw = mpool.tile([128, N], f32, tag="outw")
                nc.vector.tensor_tensor(outw[:], ktf, expm[:], op=OP.mult)
                nc.vector.tensor_reduce(
                    outT_t[:, TB * p:TB * (p + 1)],
                    outw[:].rearrange("p (b s) -> p b s", s=S),
                    axis=mybir.AxisListType.X, op=OP.add)

            nc.sync.dma_start(outT_d[:], outT_t[:])

    nc.compile()
    return nc


def _get_program():
    if "nc" not in _PROG:
        _PROG["nc"] = _build_program()
    return _PROG["nc"]


def _np_mm_dtype():
    if MM_DTYPE == "bfloat16":
        import ml_dtypes
        return np.dtype(ml_dtypes.bfloat16)
    return np.dtype(np.float32)


def kernel(query, keys, keys_length, W0, b0, W1, b1, W2, b2):
    from concourse.bass_utils import run_bass_kernel_spmd

    query = np.asarray(query, dtype=np.float32)
    keys = np.asarray(keys, dtype=np.float32)
    keys_length = np.asarray(keys_length)
    W0 = np.asarray(W0, dtype=np.float32)
    b0 = np.asarray(b0, dtype=np.float32)
    W1 = np.asarray(W1, dtype=np.float32)
    b1 = np.asarray(b1, dtype=np.float32)
    W2 = np.asarray(W2, dtype=np.float32)
    b2 = np.asarray(b2, dtype=np.float32)
    npdt = _np_mm_dtype()

    # ---- host-side weight folding (exact algebra) ----
    Wq = W0[0:E] + W0[3 * E:4 * E]
    Wkk = W0[E:2 * E] - W0[3 * E:4 * E]
    Wc = W0[2 * E:3 * E]
    wq2 = np.concatenate([Wq, Wq], 0)
    wk2 = np.concatenate([Wkk, Wkk], 0)
    wc2 = np.concatenate([Wc, Wc], 0)
    w1p = np.zeros((128, H2), np.float32)
    w1p[:] = W1
    w2r = np.repeat(W2, H2, axis=1)
    w2r2 = np.concatenate([w2r, w2r], 0)
    wpack = np.ascontiguousarray(np.concatenate(
        [wq2, wk2, wc2, w1p, w2r2], axis=1).astype(npdt))
    bpack = np.zeros((128, 3), np.float32)
    bpack[:, 0] = b0
    bpack[:, 1] = np.concatenate([b1, b1])
    bpack[:, 2] = float(b2.reshape(-1)[0])

    mask_full = (np.arange(S)[None, :] < keys_length[:, None])      # [B,S]

    nc = _get_program()

    in_maps = []
    for c in range(NCORES):
        kc = keys[c * BC:(c + 1) * BC] * mask_full[c * BC:(c + 1) * BC, :, None]
        # [1024,100,64] -> feature-major pair-packed [128, NP*400]
        kt = kc.transpose(2, 0, 1).reshape(E, BC * S)               # [64, 102400]
        kTF = np.ascontiguousarray(
            kt.reshape(E, NP, 2, N).transpose(2, 0, 1, 3).reshape(128, NP * N))
        kTP = kTF.astype(npdt)
        qc = query[c * BC:(c + 1) * BC]
        qlP = np.ascontiguousarray(
            qc.reshape(NP, 2, TB, E).transpose(0, 1, 3, 2).reshape(NP, 128, TB)
        ).astype(npdt)
        in_maps.append({"kTP": kTP, "kTF": kTF, "qlP": qlP,
                        "wpack": wpack, "bpack": bpack})

    bkr = run_bass_kernel_spmd(nc, in_maps, list(range(NCORES)))
    _PROG["last_results"] = bkr
    res = bkr.results

    out = np.empty((B, E), dtype=np.float32)
    attn = np.empty((B, S), dtype=np.float32)
    for c in range(NCORES):
        em = res[c]["em2"].reshape(BC, S).astype(np.float64)
        m = mask_full[c * BC:(c + 1) * BC]
        em = em * m
        sums = em.sum(1, keepdims=True)                              # [BC,1]
        attn[c * BC:(c + 1) * BC] = (em / sums).astype(np.float32)
        oT = res[c]["outT"]                                          # [128, 512]
        o = oT.reshape(2, E, NP, TB).transpose(2, 0, 3, 1).reshape(BC, E)
        out[c * BC:(c + 1) * BC] = (o / sums).astype(np.float32)
    return out, attn
